# revision 2
# baseline (speedup 1.0000x reference)
"""Multi-head attention Trainium2 Bass kernel, v2 (bf16 + 3-engine exp).

Structure follows the fp32r baseline (weights-stationary projections,
per-head S^T with K=64, ones-column PV for the softmax denominator,
row-parallel out-projection, host reduction over 2 head-groups) with:
  - all matmul operands bf16 (same 1 cycle/row PE rate as fp32r at
    N>=256, but half the LDWEIGHTS/DMA/SBUF traffic)
  - exp split across three engines: ACT (table exp, bf16 out) for 11/16
    of tiles, DVE and Pool computing a Schraudolph-style exp
    (int16 = s*183.87 + C, bitcast bf16) for the rest, so the scalar
    engine is never the attention-phase bottleneck
  - PSUM->SBUF projection-output quantize copies on the Pool engine
  - normalize via single-row reciprocal + PE broadcast of the
    reciprocal + one fused DVE multiply into bf16 ot tiles
  - short PE warmup (p-state ramp) overlapping the initial DMA
"""

import numpy as np
import ml_dtypes

import concourse.bass as bass
from concourse import bacc
import concourse.mybir as mybir
import concourse.tile as tile
from concourse.bass_utils import run_bass_kernel_spmd

F32 = mybir.dt.float32
F32R = mybir.dt.float32r
BF16 = mybir.dt.bfloat16
I16 = mybir.dt.int16
NPBF16 = ml_dtypes.bfloat16
P = 128

# exp split: kt2 % EXP_DVE_MOD == EXP_DVE_MOD-1 goes to DVE Schraudolph,
# the rest to ACT table exp (GPSIMD cannot read PSUM, so no Pool exp)
EXP_DVE_MOD = 4

# Schraudolph bf16: i16 = round(s_eff*(2^7/ln2) + C16); bitcast bf16
SCH_A16 = float(128.0 / np.log(2.0))
SCH_C16 = 16250.5  # tuned offline; adjust after HW comparison


def _install_drain_patch():
    import concourse.tile as _tile
    import concourse.mybir as _mybir
    from concourse.vector_clock import ScopedClock as _ScopedClock

    if getattr(_tile.TileContext, "_drain_split_patch", False):
        return

    def _patched_drain_and_barrier(self, tick_clock, wait_clock):
        drain_inst = self.nc.sync.drain()
        wait_clock.add_sem_waits(
            drain_inst.ins, _ScopedClock({None: tick_clock.global_clock})
        )
        si = drain_inst.ins.sync_info
        if si is not None and len(si.on_wait) > 1:
            waits = list(si.on_wait)
            drain_inst.ins.sync_info = _mybir.SyncInfo(
                on_wait=[waits[0]], on_update=list(si.on_update)
            )
            for w in waits[1:]:
                extra = self.nc.sync.drain()
                extra.ins.sync_info = _mybir.SyncInfo(on_wait=[w], on_update=[])
        self.nc.all_engine_barrier()
        assert self.sems is not None
        popped = self.nc._tile_sem_poison_stack.pop()
        assert popped is self._sem_poison
        self.nc.clear_and_free_semaphores(list(self.sems.allocated().values()))
        self.nc.all_engine_barrier()

    _tile.TileContext._drain_and_barrier = _patched_drain_and_barrier
    _tile.TileContext._drain_split_patch = True


_install_drain_patch()


def build_core_program(
    nq=2048,
    nkv=2048,
    ckdim=1024,
    n_heads=8,
    hd=64,
    odim=1024,
    cs=512,
    num_devices=8,
    warmup=90,
    exp_dve_mod=EXP_DVE_MOD,
):
    d_local = n_heads * hd  # 512
    scale = float(hd) ** -0.5
    sch_a = float(SCH_A16 * scale)
    KC = ckdim // P  # 8 contraction tiles for projections
    DMT = d_local // P  # 4 partition tiles (head pairs)
    NQC = nq // cs  # 4
    NVC = nkv // cs  # 4
    NKT = nkv // P  # 16 kv tiles
    OT = odim // P  # 8
    SUB = cs // P  # 4

    nc = bacc.Bacc(
        "TRN2", target_bir_lowering=False, debug=False, num_devices=num_devices
    )
    xqT = nc.declare_dram_parameter("xqT", [ckdim, nq], BF16, isOutput=False)
    xkT = nc.declare_dram_parameter("xkT", [ckdim, nkv], BF16, isOutput=False)
    xvT = nc.declare_dram_parameter("xvT", [ckdim, nkv], BF16, isOutput=False)
    wqT = nc.declare_dram_parameter("wqT", [ckdim, d_local], BF16, isOutput=False)
    wkT = nc.declare_dram_parameter("wkT", [ckdim, d_local], BF16, isOutput=False)
    wvT = nc.declare_dram_parameter("wvT", [ckdim, d_local], BF16, isOutput=False)
    woT = nc.declare_dram_parameter("woT", [d_local, odim], BF16, isOutput=False)
    outT = nc.declare_dram_parameter("outT", [odim, nq], F32, isOutput=True)

    with tile.TileContext(nc) as tc:
        import contextlib

        ctx = contextlib.ExitStack()
        with ctx:
            ctx.enter_context(
                nc.allow_low_precision(reason="bf16 pipeline, fp32 PSUM accum")
            )
            w_pool = ctx.enter_context(tc.tile_pool(name="w", bufs=1))
            xin_pool = ctx.enter_context(tc.tile_pool(name="xin", bufs=10))
            qt_pool = ctx.enter_context(tc.tile_pool(name="qt", bufs=1))
            kt_pool = ctx.enter_context(tc.tile_pool(name="kt", bufs=1))
            v_pool = ctx.enter_context(tc.tile_pool(name="v", bufs=1))
            ex_pool = ctx.enter_context(tc.tile_pool(name="expp", bufs=8))
            ot_pool = ctx.enter_context(tc.tile_pool(name="ot", bufs=2))
            rcp_pool = ctx.enter_context(tc.tile_pool(name="rcp", bufs=4))
            out_pool = ctx.enter_context(tc.tile_pool(name="outp", bufs=3))
            ones_pool = ctx.enter_context(tc.tile_pool(name="ones", bufs=1))
            ps_a = ctx.enter_context(tc.tile_pool(name="ps_a", bufs=2, space="PSUM"))
            ps_ot = ctx.enter_context(tc.tile_pool(name="ps_ot", bufs=2, space="PSUM"))
            ps_st = ctx.enter_context(tc.tile_pool(name="ps_st", bufs=2, space="PSUM"))

            # ---- resident weights (bf16) -------------------------------
            def load_w(dram, label):
                tiles = []
                for kc in range(KC):
                    t = w_pool.tile([P, d_local], BF16, name=f"{label}{kc}")
                    nc.sync.dma_start(out=t, in_=dram[kc * P : (kc + 1) * P, :])
                    tiles.append(t)
                return tiles

            wq_sb = load_w(wqT, "wq")
            wk_sb = load_w(wkT, "wk")
            wv_sb = load_w(wvT, "wv")
            wo_sb = []
            for mt in range(DMT):
                t = w_pool.tile([P, odim], BF16, name=f"wo{mt}")
                nc.sync.dma_start(out=t, in_=woT[mt * P : (mt + 1) * P, :])
                wo_sb.append(t)

            ones_t = ones_pool.tile([P, hd], BF16, name="ones_t")
            nc.vector.memset(ones_t, 1.0)
            warm_t = ones_pool.tile([hd, hd], BF16, name="warm_t")
            nc.vector.memset(warm_t, 1.0)

            warm = ps_a.tile([hd, hd], F32, name="warm", tag="psa")
            for _ in range(warmup):
                nc.tensor.matmul(warm, lhsT=warm_t, rhs=warm_t, start=True, stop=True)

            # ---- persistent attention operand tiles --------------------
            qt_sb = [
                [qt_pool.tile([P, cs], BF16, name=f"qt{mt}_{c}") for c in range(NQC)]
                for mt in range(DMT)
            ]
            kt_sb = [
                [kt_pool.tile([P, cs], BF16, name=f"kt{mt}_{c}") for c in range(NVC)]
                for mt in range(DMT)
            ]
            v_t = [
                v_pool.tile([P, n_heads, hd + 1], BF16, name=f"v{nt}")
                for nt in range(NKT)
            ]
            for nt in range(NKT):
                nc.gpsimd.memset(v_t[nt][:, :, hd : hd + 1], 1.0)

            # ---- projections -------------------------------------------
            def project_qk(x_dram, w_sb, out_tiles, label):
                for nch in range(NQC):
                    xch = []
                    for kc in range(KC):
                        t = xin_pool.tile([P, cs], BF16, name=f"x{label}{nch}_{kc}",
                                          tag="xin")
                        nc.sync.dma_start(
                            out=t,
                            in_=x_dram[kc * P : (kc + 1) * P,
                                       nch * cs : (nch + 1) * cs],
                        )
                        xch.append(t)
                    for mt in range(DMT):
                        ps = ps_a.tile([P, cs], F32, name=f"p{label}{nch}_{mt}",
                                       tag="psa")
                        for kc in range(KC):
                            nc.tensor.matmul(
                                ps,
                                lhsT=w_sb[kc][:, mt * P : (mt + 1) * P],
                                rhs=xch[kc],
                                start=(kc == 0),
                                stop=(kc == KC - 1),
                            )
                        nc.scalar.copy(out=out_tiles[mt][nch], in_=ps)

            project_qk(xqT, wq_sb, qt_sb, "q")
            project_qk(xkT, wk_sb, kt_sb, "k")

            for nch in range(NVC):
                xch = []
                for kc in range(KC):
                    t = xin_pool.tile([P, cs], BF16, name=f"xv{nch}_{kc}", tag="xin")
                    nc.sync.dma_start(
                        out=t,
                        in_=xvT[kc * P : (kc + 1) * P, nch * cs : (nch + 1) * cs],
                    )
                    xch.append(t)
                for sub in range(SUB):
                    ps = ps_a.tile([P, d_local], F32, name=f"pv{nch}_{sub}",
                                   tag="psa")
                    for kc in range(KC):
                        nc.tensor.matmul(
                            ps,
                            lhsT=xch[kc][:, sub * P : (sub + 1) * P],
                            rhs=wv_sb[kc],
                            start=(kc == 0),
                            stop=(kc == KC - 1),
                        )
                    nt = nch * SUB + sub
                    nc.scalar.copy(
                        out=v_t[nt][:, :, 0:hd],
                        in_=ps.rearrange("p (h d) -> p h d", h=n_heads),
                    )

            # ---- attention ---------------------------------------------
            def emit_exp(ex, st, kt2):
                if kt2 % exp_dve_mod == exp_dve_mod - 1:
                    nc.vector.tensor_scalar(
                        out=ex.bitcast(I16),
                        in0=st,
                        scalar1=sch_a,
                        scalar2=SCH_C16,
                        op0=mybir.AluOpType.mult,
                        op1=mybir.AluOpType.add,
                    )
                else:
                    nc.scalar.activation(
                        out=ex,
                        in_=st,
                        func=mybir.ActivationFunctionType.Exp,
                        scale=scale,
                    )

            for qg in range(NQC):
                qsl = slice(qg * cs, (qg + 1) * cs)
                ot_t = ot_pool.tile([P, DMT, cs], BF16, name=f"ot{qg}", tag="ot")
                for hp in range(DMT):
                    heads = (2 * hp, 2 * hp + 1)
                    ot_ps = {
                        h: ps_ot.tile([hd + 1, cs], F32, name=f"otps{qg}_{h}",
                                      tag="psot")
                        for h in heads
                    }

                    def emit_pv(kt2_p, ex_p):
                        for jk in range(2):
                            kt = 2 * kt2_p + jk
                            for h in heads:
                                nc.tensor.matmul(
                                    ot_ps[h],
                                    lhsT=v_t[kt][:, h, :],
                                    rhs=ex_p[h][:, jk, :],
                                    start=(kt == 0),
                                    stop=(kt == NKT - 1),
                                )

                    pending = None
                    for kt2 in range(NKT // 2):
                        st = {
                            h: ps_st.tile([P, 2, cs], F32,
                                          name=f"st{qg}_{hp}_{kt2}_{h}", tag="st")
                            for h in heads
                        }
                        for jk in range(2):
                            kt = 2 * kt2 + jk
                            kch, sub = divmod(kt, SUB)
                            for h in heads:
                                poff = (h % 2) * hd
                                nc.tensor.matmul(
                                    st[h][:, jk, :],
                                    lhsT=kt_sb[hp][kch][
                                        poff : poff + hd,
                                        sub * P : (sub + 1) * P,
                                    ],
                                    rhs=qt_sb[hp][qg][poff : poff + hd, :],
                                    start=True,
                                    stop=True,
                                )
                        ex = {}
                        for h in heads:
                            ex[h] = ex_pool.tile([P, 2, cs], BF16,
                                                 name=f"ex{qg}_{kt2}_{h}", tag="ex")
                            emit_exp(ex[h], st[h], kt2)
                        if pending is not None:
                            emit_pv(*pending)
                        pending = (kt2, ex)
                    emit_pv(*pending)

                    for h in heads:
                        poff = (h % 2) * hd
                        raw = rcp_pool.tile([hd + 1, cs], BF16, name=f"raw{qg}_{h}",
                                            tag="raw")
                        nc.scalar.copy(out=raw, in_=ot_ps[h])
                        bcp = ps_a.tile([hd, cs], F32, name=f"bcp{qg}_{h}",
                                        tag="psa")
                        nc.tensor.matmul(
                            bcp,
                            lhsT=ones_t[hd : hd + 1, :],
                            rhs=raw[hd : hd + 1, :],
                            start=True,
                            stop=True,
                        )
                        bci = rcp_pool.tile([hd, cs], F32, name=f"bci{qg}_{h}",
                                            tag="bci")
                        nc.vector.reciprocal_approx_fast(out=bci, in_=bcp)
                        nc.vector.scalar_tensor_tensor(
                            out=ot_t[poff : poff + hd, hp, :],
                            in0=raw[0:hd, :],
                            scalar=1.0,
                            in1=bci,
                            op0=mybir.AluOpType.mult,
                            op1=mybir.AluOpType.mult,
                        )

                for ob in range(OT):
                    po = ps_a.tile([P, cs], F32, name=f"po{qg}_{ob}", tag="psa")
                    for hp in range(DMT):
                        nc.tensor.matmul(
                            po,
                            lhsT=wo_sb[hp][:, ob * P : (ob + 1) * P],
                            rhs=ot_t[:, hp, :],
                            start=(hp == 0),
                            stop=(hp == DMT - 1),
                        )
                    osb = out_pool.tile([P, cs], F32, name=f"osb{qg}_{ob}",
                                        tag="osb")
                    nc.vector.tensor_copy(out=osb, in_=po)
                    nc.sync.dma_start(out=outT[ob * P : (ob + 1) * P, qsl], in_=osb)
    nc.finalize()
    return nc


_NC_CACHE = {}


def _get_program(key, **kw):
    if key not in _NC_CACHE:
        _NC_CACHE[key] = build_core_program(**kw)
    return _NC_CACHE[key]


def _bf16(a):
    return np.ascontiguousarray(a.astype(NPBF16))


def kernel(xq, xk, xv, Wq, Wk, Wv, Wo, bo):
    m, nq, qkd = xq.shape
    nkv = xk.shape[1]
    inner = Wq.shape[0]
    odim = Wo.shape[0]
    assert (m, nq, qkd, nkv, inner, odim) == (4, 2048, 1024, 2048, 1024, 1024)
    n_cores = 8
    gheads = 2
    gslice = inner // gheads  # 512

    WqT = np.asarray(Wq, np.float32).T
    WkT = np.asarray(Wk, np.float32).T
    WvT = np.asarray(Wv, np.float32).T
    WoT = np.asarray(Wo, np.float32).T

    xq_b = [_bf16(np.asarray(xq[b], np.float32).T) for b in range(m)]
    xk_b = [_bf16(np.asarray(xk[b], np.float32).T) for b in range(m)]
    xv_b = [_bf16(np.asarray(xv[b], np.float32).T) for b in range(m)]

    in_maps = []
    for c in range(n_cores):
        b, g = divmod(c, gheads)
        sl = slice(g * gslice, (g + 1) * gslice)
        in_maps.append(
            {
                "xqT": xq_b[b],
                "xkT": xk_b[b],
                "xvT": xv_b[b],
                "wqT": _bf16(WqT[:, sl]),
                "wkT": _bf16(WkT[:, sl]),
                "wvT": _bf16(WvT[:, sl]),
                "woT": _bf16(WoT[sl, :]),
            }
        )

    nc = _get_program("full")
    res = run_bass_kernel_spmd(nc, in_maps, core_ids=list(range(n_cores)))
    global _LAST_RESULTS
    _LAST_RESULTS = res
    out = np.empty((m, nq, odim), np.float32)
    bo32 = np.asarray(bo, np.float32)[None, :]
    for b in range(m):
        acc = res.results[gheads * b]["outT"].copy()
        for g in range(1, gheads):
            acc += res.results[gheads * b + g]["outT"]
        out[b] = acc.T + bo32
    return out


# revision 3
# speedup vs baseline: 1.0269x; 1.0269x over previous
"""Multi-head attention Trainium2 Bass kernel, v2 (bf16 + 3-engine exp).

Structure follows the fp32r baseline (weights-stationary projections,
per-head S^T with K=64, ones-column PV for the softmax denominator,
row-parallel out-projection, host reduction over 2 head-groups) with:
  - all matmul operands bf16 (same 1 cycle/row PE rate as fp32r at
    N>=256, but half the LDWEIGHTS/DMA/SBUF traffic)
  - exp split across three engines: ACT (table exp, bf16 out) for 11/16
    of tiles, DVE and Pool computing a Schraudolph-style exp
    (int16 = s*183.87 + C, bitcast bf16) for the rest, so the scalar
    engine is never the attention-phase bottleneck
  - PSUM->SBUF projection-output quantize copies on the Pool engine
  - normalize via single-row reciprocal + PE broadcast of the
    reciprocal + one fused DVE multiply into bf16 ot tiles
  - short PE warmup (p-state ramp) overlapping the initial DMA
"""

import numpy as np
import ml_dtypes

import concourse.bass as bass
from concourse import bacc
import concourse.mybir as mybir
import concourse.tile as tile
from concourse.bass_utils import run_bass_kernel_spmd

F32 = mybir.dt.float32
F32R = mybir.dt.float32r
BF16 = mybir.dt.bfloat16
I16 = mybir.dt.int16
NPBF16 = ml_dtypes.bfloat16
P = 128

# exp split: kt2 % EXP_DVE_MOD == EXP_DVE_MOD-1 goes to DVE Schraudolph,
# the rest to ACT table exp (GPSIMD cannot read PSUM, so no Pool exp)
EXP_DVE_MOD = 4

# Schraudolph bf16: i16 = round(s_eff*(2^7/ln2) + C16); bitcast bf16
SCH_A16 = float(128.0 / np.log(2.0))
SCH_C16 = 16250.5  # tuned offline; adjust after HW comparison


def _install_drain_patch():
    import concourse.tile as _tile
    import concourse.mybir as _mybir
    from concourse.vector_clock import ScopedClock as _ScopedClock

    if getattr(_tile.TileContext, "_drain_split_patch", False):
        return

    def _patched_drain_and_barrier(self, tick_clock, wait_clock):
        drain_inst = self.nc.sync.drain()
        wait_clock.add_sem_waits(
            drain_inst.ins, _ScopedClock({None: tick_clock.global_clock})
        )
        si = drain_inst.ins.sync_info
        if si is not None and len(si.on_wait) > 1:
            waits = list(si.on_wait)
            drain_inst.ins.sync_info = _mybir.SyncInfo(
                on_wait=[waits[0]], on_update=list(si.on_update)
            )
            for w in waits[1:]:
                extra = self.nc.sync.drain()
                extra.ins.sync_info = _mybir.SyncInfo(on_wait=[w], on_update=[])
        self.nc.all_engine_barrier()
        assert self.sems is not None
        popped = self.nc._tile_sem_poison_stack.pop()
        assert popped is self._sem_poison
        self.nc.clear_and_free_semaphores(list(self.sems.allocated().values()))
        self.nc.all_engine_barrier()

    _tile.TileContext._drain_and_barrier = _patched_drain_and_barrier
    _tile.TileContext._drain_split_patch = True


_install_drain_patch()


def build_core_program(
    nq=2048,
    nkv=2048,
    ckdim=1024,
    n_heads=8,
    hd=64,
    odim=1024,
    cs=512,
    num_devices=8,
    warmup=90,
    exp_dve_mod=EXP_DVE_MOD,
):
    d_local = n_heads * hd  # 512
    scale = float(hd) ** -0.5
    sch_a = float(SCH_A16 * scale)
    KC = ckdim // P  # 8 contraction tiles for projections
    DMT = d_local // P  # 4 partition tiles (head pairs)
    NQC = nq // cs  # 4
    NVC = nkv // cs  # 4
    NKT = nkv // P  # 16 kv tiles
    OT = odim // P  # 8
    SUB = cs // P  # 4

    nc = bacc.Bacc(
        "TRN2", target_bir_lowering=False, debug=False, num_devices=num_devices
    )
    xqT = nc.declare_dram_parameter("xqT", [ckdim, nq], BF16, isOutput=False)
    xkT = nc.declare_dram_parameter("xkT", [ckdim, nkv], BF16, isOutput=False)
    xvT = nc.declare_dram_parameter("xvT", [ckdim, nkv], BF16, isOutput=False)
    wqT = nc.declare_dram_parameter("wqT", [ckdim, d_local], BF16, isOutput=False)
    wkT = nc.declare_dram_parameter("wkT", [ckdim, d_local], BF16, isOutput=False)
    wvT = nc.declare_dram_parameter("wvT", [ckdim, d_local], BF16, isOutput=False)
    woT = nc.declare_dram_parameter("woT", [d_local, odim], BF16, isOutput=False)
    outT = nc.declare_dram_parameter("outT", [odim, nq], F32, isOutput=True)

    with tile.TileContext(nc) as tc:
        import contextlib

        ctx = contextlib.ExitStack()
        with ctx:
            ctx.enter_context(
                nc.allow_low_precision(reason="bf16 pipeline, fp32 PSUM accum")
            )
            w_pool = ctx.enter_context(tc.tile_pool(name="w", bufs=1))
            xin_pool = ctx.enter_context(tc.tile_pool(name="xin", bufs=10))
            qt_pool = ctx.enter_context(tc.tile_pool(name="qt", bufs=1))
            kt_pool = ctx.enter_context(tc.tile_pool(name="kt", bufs=1))
            v_pool = ctx.enter_context(tc.tile_pool(name="v", bufs=1))
            ex_pool = ctx.enter_context(tc.tile_pool(name="expp", bufs=12))
            ot_pool = ctx.enter_context(tc.tile_pool(name="ot", bufs=2))
            rcp_pool = ctx.enter_context(tc.tile_pool(name="rcp", bufs=4))
            out_pool = ctx.enter_context(tc.tile_pool(name="outp", bufs=3))
            ones_pool = ctx.enter_context(tc.tile_pool(name="ones", bufs=1))
            ps_a = ctx.enter_context(tc.tile_pool(name="ps_a", bufs=2, space="PSUM"))
            ps_ot = ctx.enter_context(tc.tile_pool(name="ps_ot", bufs=2, space="PSUM"))
            ps_st = ctx.enter_context(tc.tile_pool(name="ps_st", bufs=2, space="PSUM"))

            # ---- resident weights (bf16) -------------------------------
            def load_w(dram, label):
                tiles = []
                for kc in range(KC):
                    t = w_pool.tile([P, d_local], BF16, name=f"{label}{kc}")
                    nc.sync.dma_start(out=t, in_=dram[kc * P : (kc + 1) * P, :])
                    tiles.append(t)
                return tiles

            wq_sb = load_w(wqT, "wq")

            ones_t = ones_pool.tile([P, hd], BF16, name="ones_t")
            nc.vector.memset(ones_t, 1.0)
            warm_t = ones_pool.tile([hd, hd], BF16, name="warm_t")
            nc.vector.memset(warm_t, 1.0)

            warm = ps_a.tile([hd, hd], F32, name="warm", tag="psa")
            for _ in range(warmup):
                nc.tensor.matmul(warm, lhsT=warm_t, rhs=warm_t, start=True, stop=True)

            # ---- persistent attention operand tiles --------------------
            qt_sb = [
                [qt_pool.tile([P, cs], BF16, name=f"qt{mt}_{c}") for c in range(NQC)]
                for mt in range(DMT)
            ]
            kt_sb = [
                [kt_pool.tile([P, cs], BF16, name=f"kt{mt}_{c}") for c in range(NVC)]
                for mt in range(DMT)
            ]
            v_t = [
                v_pool.tile([P, n_heads, hd + 1], BF16, name=f"v{nt}")
                for nt in range(NKT)
            ]
            for nt in range(NKT):
                nc.gpsimd.memset(v_t[nt][:, :, hd : hd + 1], 1.0)

            # ---- projections -------------------------------------------
            def project_qk(x_dram, w_sb, out_tiles, label):
                for nch in range(NQC):
                    xch = []
                    for kc in range(KC):
                        t = xin_pool.tile([P, cs], BF16, name=f"x{label}{nch}_{kc}",
                                          tag="xin")
                        nc.sync.dma_start(
                            out=t,
                            in_=x_dram[kc * P : (kc + 1) * P,
                                       nch * cs : (nch + 1) * cs],
                        )
                        xch.append(t)
                    for mt in range(DMT):
                        ps = ps_a.tile([P, cs], F32, name=f"p{label}{nch}_{mt}",
                                       tag="psa")
                        for kc in range(KC):
                            nc.tensor.matmul(
                                ps,
                                lhsT=w_sb[kc][:, mt * P : (mt + 1) * P],
                                rhs=xch[kc],
                                start=(kc == 0),
                                stop=(kc == KC - 1),
                            )
                        nc.scalar.copy(out=out_tiles[mt][nch], in_=ps)

            project_qk(xqT, wq_sb, qt_sb, "q")
            wk_sb = load_w(wkT, "wk")
            project_qk(xkT, wk_sb, kt_sb, "k")
            wv_sb = load_w(wvT, "wv")

            for nch in range(NVC):
                xch = []
                for kc in range(KC):
                    t = xin_pool.tile([P, cs], BF16, name=f"xv{nch}_{kc}", tag="xin")
                    nc.sync.dma_start(
                        out=t,
                        in_=xvT[kc * P : (kc + 1) * P, nch * cs : (nch + 1) * cs],
                    )
                    xch.append(t)
                for sub in range(SUB):
                    ps = ps_a.tile([P, d_local], F32, name=f"pv{nch}_{sub}",
                                   tag="psa")
                    for kc in range(KC):
                        nc.tensor.matmul(
                            ps,
                            lhsT=xch[kc][:, sub * P : (sub + 1) * P],
                            rhs=wv_sb[kc],
                            start=(kc == 0),
                            stop=(kc == KC - 1),
                        )
                    nt = nch * SUB + sub
                    nc.scalar.copy(
                        out=v_t[nt][:, :, 0:hd],
                        in_=ps.rearrange("p (h d) -> p h d", h=n_heads),
                    )

            wo_sb = []
            for mt in range(DMT):
                t = w_pool.tile([P, odim], BF16, name=f"wo{mt}")
                nc.sync.dma_start(out=t, in_=woT[mt * P : (mt + 1) * P, :])
                wo_sb.append(t)

            # ---- attention ---------------------------------------------
            def emit_exp(ex, st, kt2):
                if kt2 % exp_dve_mod == exp_dve_mod - 1:
                    nc.vector.tensor_scalar(
                        out=ex.bitcast(I16),
                        in0=st,
                        scalar1=sch_a,
                        scalar2=SCH_C16,
                        op0=mybir.AluOpType.mult,
                        op1=mybir.AluOpType.add,
                    )
                else:
                    nc.scalar.activation(
                        out=ex,
                        in_=st,
                        func=mybir.ActivationFunctionType.Exp,
                        scale=scale,
                    )

            SKEW = 2
            for qg in range(NQC):
                qsl = slice(qg * cs, (qg + 1) * cs)
                ot_t = ot_pool.tile([P, DMT, cs], BF16, name=f"ot{qg}", tag="ot")
                prev_norm = None

                def make_norm(ot_ps_n, heads_n, hp_n):
                    def norm():
                        for h in heads_n:
                            poff = (h % 2) * hd
                            raw = rcp_pool.tile([hd + 1, cs], BF16,
                                                name=f"raw{qg}_{h}", tag="raw")
                            nc.scalar.copy(out=raw, in_=ot_ps_n[h])
                            bcp = ps_a.tile([hd, cs], F32, name=f"bcp{qg}_{h}",
                                            tag="psa")
                            nc.tensor.matmul(
                                bcp,
                                lhsT=ones_t[hd : hd + 1, :],
                                rhs=raw[hd : hd + 1, :],
                                start=True,
                                stop=True,
                            )
                            bci = rcp_pool.tile([hd, cs], F32,
                                                name=f"bci{qg}_{h}", tag="bci")
                            nc.vector.reciprocal_approx_fast(out=bci, in_=bcp)
                            nc.vector.scalar_tensor_tensor(
                                out=ot_t[poff : poff + hd, hp_n, :],
                                in0=raw[0:hd, :],
                                scalar=1.0,
                                in1=bci,
                                op0=mybir.AluOpType.mult,
                                op1=mybir.AluOpType.mult,
                            )
                    return norm

                for hp in range(DMT):
                    heads = (2 * hp, 2 * hp + 1)
                    ot_ps = {
                        h: ps_ot.tile([hd + 1, cs], F32, name=f"otps{qg}_{h}",
                                      tag="psot")
                        for h in heads
                    }

                    def emit_pv(kt2_p, ex_p, heads_p=heads, ot_ps_p=ot_ps):
                        for jk in range(2):
                            kt = 2 * kt2_p + jk
                            for h in heads_p:
                                nc.tensor.matmul(
                                    ot_ps_p[h],
                                    lhsT=v_t[kt][:, h, :],
                                    rhs=ex_p[h][:, jk, :],
                                    start=(kt == 0),
                                    stop=(kt == NKT - 1),
                                )

                    pend = []
                    for kt2 in range(NKT // 2):
                        st = {
                            h: ps_st.tile([P, 2, cs], F32,
                                          name=f"st{qg}_{hp}_{kt2}_{h}", tag="st")
                            for h in heads
                        }
                        for jk in range(2):
                            kt = 2 * kt2 + jk
                            kch, sub = divmod(kt, SUB)
                            for h in heads:
                                poff = (h % 2) * hd
                                nc.tensor.matmul(
                                    st[h][:, jk, :],
                                    lhsT=kt_sb[hp][kch][
                                        poff : poff + hd,
                                        sub * P : (sub + 1) * P,
                                    ],
                                    rhs=qt_sb[hp][qg][poff : poff + hd, :],
                                    start=True,
                                    stop=True,
                                )
                        ex = {}
                        for h in heads:
                            ex[h] = ex_pool.tile([P, 2, cs], BF16,
                                                 name=f"ex{qg}_{kt2}_{h}", tag="ex")
                            emit_exp(ex[h], st[h], kt2)
                        if kt2 == 1 and prev_norm is not None:
                            prev_norm()
                        pend.append((kt2, ex))
                        if len(pend) > SKEW:
                            emit_pv(*pend.pop(0))
                    for p in pend:
                        emit_pv(*p)
                    prev_norm = make_norm(ot_ps, heads, hp)
                prev_norm()

                for ob in range(OT):
                    po = ps_a.tile([P, cs], F32, name=f"po{qg}_{ob}", tag="psa")
                    for hp in range(DMT):
                        nc.tensor.matmul(
                            po,
                            lhsT=wo_sb[hp][:, ob * P : (ob + 1) * P],
                            rhs=ot_t[:, hp, :],
                            start=(hp == 0),
                            stop=(hp == DMT - 1),
                        )
                    osb = out_pool.tile([P, cs], F32, name=f"osb{qg}_{ob}",
                                        tag="osb")
                    nc.vector.tensor_copy(out=osb, in_=po)
                    nc.sync.dma_start(out=outT[ob * P : (ob + 1) * P, qsl], in_=osb)
    nc.finalize()
    return nc


_NC_CACHE = {}


def _get_program(key, **kw):
    if key not in _NC_CACHE:
        _NC_CACHE[key] = build_core_program(**kw)
    return _NC_CACHE[key]


def _bf16(a):
    return np.ascontiguousarray(a.astype(NPBF16))


def kernel(xq, xk, xv, Wq, Wk, Wv, Wo, bo):
    m, nq, qkd = xq.shape
    nkv = xk.shape[1]
    inner = Wq.shape[0]
    odim = Wo.shape[0]
    assert (m, nq, qkd, nkv, inner, odim) == (4, 2048, 1024, 2048, 1024, 1024)
    n_cores = 8
    gheads = 2
    gslice = inner // gheads  # 512

    WqT = np.asarray(Wq, np.float32).T
    WkT = np.asarray(Wk, np.float32).T
    WvT = np.asarray(Wv, np.float32).T
    WoT = np.asarray(Wo, np.float32).T

    xq_b = [_bf16(np.asarray(xq[b], np.float32).T) for b in range(m)]
    xk_b = [_bf16(np.asarray(xk[b], np.float32).T) for b in range(m)]
    xv_b = [_bf16(np.asarray(xv[b], np.float32).T) for b in range(m)]

    in_maps = []
    for c in range(n_cores):
        b, g = divmod(c, gheads)
        sl = slice(g * gslice, (g + 1) * gslice)
        in_maps.append(
            {
                "xqT": xq_b[b],
                "xkT": xk_b[b],
                "xvT": xv_b[b],
                "wqT": _bf16(WqT[:, sl]),
                "wkT": _bf16(WkT[:, sl]),
                "wvT": _bf16(WvT[:, sl]),
                "woT": _bf16(WoT[sl, :]),
            }
        )

    nc = _get_program("full")
    res = run_bass_kernel_spmd(nc, in_maps, core_ids=list(range(n_cores)))
    global _LAST_RESULTS
    _LAST_RESULTS = res
    out = np.empty((m, nq, odim), np.float32)
    bo32 = np.asarray(bo, np.float32)[None, :]
    for b in range(m):
        acc = res.results[gheads * b]["outT"].copy()
        for g in range(1, gheads):
            acc += res.results[gheads * b + g]["outT"]
        out[b] = acc.T + bo32
    return out


# revision 4
# speedup vs baseline: 1.0671x; 1.0391x over previous
"""Multi-head attention Trainium2 Bass kernel, v2 (bf16 + 3-engine exp).

Structure follows the fp32r baseline (weights-stationary projections,
per-head S^T with K=64, ones-column PV for the softmax denominator,
row-parallel out-projection, host reduction over 2 head-groups) with:
  - all matmul operands bf16 (same 1 cycle/row PE rate as fp32r at
    N>=256, but half the LDWEIGHTS/DMA/SBUF traffic)
  - exp split across three engines: ACT (table exp, bf16 out) for 11/16
    of tiles, DVE and Pool computing a Schraudolph-style exp
    (int16 = s*183.87 + C, bitcast bf16) for the rest, so the scalar
    engine is never the attention-phase bottleneck
  - PSUM->SBUF projection-output quantize copies on the Pool engine
  - normalize via single-row reciprocal + PE broadcast of the
    reciprocal + one fused DVE multiply into bf16 ot tiles
  - short PE warmup (p-state ramp) overlapping the initial DMA
"""

import numpy as np
import ml_dtypes

import concourse.bass as bass
from concourse import bacc
import concourse.mybir as mybir
import concourse.tile as tile
from concourse.bass_utils import run_bass_kernel_spmd

F32 = mybir.dt.float32
F32R = mybir.dt.float32r
BF16 = mybir.dt.bfloat16
I16 = mybir.dt.int16
NPBF16 = ml_dtypes.bfloat16
P = 128

# exp split: kt2 % EXP_DVE_MOD == EXP_DVE_MOD-1 goes to DVE Schraudolph,
# the rest to ACT table exp (GPSIMD cannot read PSUM, so no Pool exp)
EXP_DVE_MOD = 4

# Schraudolph bf16: i16 = round(s_eff*(2^7/ln2) + C16); bitcast bf16
SCH_A16 = float(128.0 / np.log(2.0))
SCH_C16 = 16250.5  # tuned offline; adjust after HW comparison


def _install_drain_patch():
    import concourse.tile as _tile
    import concourse.mybir as _mybir
    from concourse.vector_clock import ScopedClock as _ScopedClock

    if getattr(_tile.TileContext, "_drain_split_patch", False):
        return

    def _patched_drain_and_barrier(self, tick_clock, wait_clock):
        drain_inst = self.nc.sync.drain()
        wait_clock.add_sem_waits(
            drain_inst.ins, _ScopedClock({None: tick_clock.global_clock})
        )
        si = drain_inst.ins.sync_info
        if si is not None and len(si.on_wait) > 1:
            waits = list(si.on_wait)
            drain_inst.ins.sync_info = _mybir.SyncInfo(
                on_wait=[waits[0]], on_update=list(si.on_update)
            )
            for w in waits[1:]:
                extra = self.nc.sync.drain()
                extra.ins.sync_info = _mybir.SyncInfo(on_wait=[w], on_update=[])
        self.nc.all_engine_barrier()
        assert self.sems is not None
        popped = self.nc._tile_sem_poison_stack.pop()
        assert popped is self._sem_poison
        self.nc.clear_and_free_semaphores(list(self.sems.allocated().values()))
        self.nc.all_engine_barrier()

    _tile.TileContext._drain_and_barrier = _patched_drain_and_barrier
    _tile.TileContext._drain_split_patch = True


_install_drain_patch()


def build_core_program(
    nq=2048,
    nkv=2048,
    ckdim=1024,
    n_heads=8,
    hd=64,
    odim=1024,
    cs=512,
    num_devices=8,
    warmup=130,
    exp_dve_mod=EXP_DVE_MOD,
):
    d_local = n_heads * hd  # 512
    scale = float(hd) ** -0.5
    sch_a = float(SCH_A16 * scale)
    KC = ckdim // P  # 8 contraction tiles for projections
    DMT = d_local // P  # 4 partition tiles (head pairs)
    NQC = nq // cs  # 4
    NVC = nkv // cs  # 4
    NKT = nkv // P  # 16 kv tiles
    OT = odim // P  # 8
    SUB = cs // P  # 4

    nc = bacc.Bacc(
        "TRN2", target_bir_lowering=False, debug=False, num_devices=num_devices
    )
    xqT = nc.declare_dram_parameter("xqT", [ckdim, nq], BF16, isOutput=False)
    xkT = nc.declare_dram_parameter("xkT", [ckdim, nkv], BF16, isOutput=False)
    xvT = nc.declare_dram_parameter("xvT", [ckdim, nkv], BF16, isOutput=False)
    wqT = nc.declare_dram_parameter("wqT", [ckdim, d_local], BF16, isOutput=False)
    wkT = nc.declare_dram_parameter("wkT", [ckdim, d_local], BF16, isOutput=False)
    wvT = nc.declare_dram_parameter("wvT", [ckdim, d_local], BF16, isOutput=False)
    woT = nc.declare_dram_parameter("woT", [d_local, odim], BF16, isOutput=False)
    outT = nc.declare_dram_parameter("outT", [odim, nq], F32, isOutput=True)

    with tile.TileContext(nc) as tc:
        import contextlib

        ctx = contextlib.ExitStack()
        with ctx:
            ctx.enter_context(
                nc.allow_low_precision(reason="bf16 pipeline, fp32 PSUM accum")
            )
            w_pool = ctx.enter_context(tc.tile_pool(name="w", bufs=1))
            xin_pool = ctx.enter_context(tc.tile_pool(name="xin", bufs=18))
            qt_pool = ctx.enter_context(tc.tile_pool(name="qt", bufs=1))
            kt_pool = ctx.enter_context(tc.tile_pool(name="kt", bufs=1))
            v_pool = ctx.enter_context(tc.tile_pool(name="v", bufs=1))
            ex_pool = ctx.enter_context(tc.tile_pool(name="expp", bufs=12))
            ot_pool = ctx.enter_context(tc.tile_pool(name="ot", bufs=2))
            rcp_pool = ctx.enter_context(tc.tile_pool(name="rcp", bufs=4))
            out_pool = ctx.enter_context(tc.tile_pool(name="outp", bufs=3))
            ones_pool = ctx.enter_context(tc.tile_pool(name="ones", bufs=1))
            ps_a = ctx.enter_context(tc.tile_pool(name="ps_a", bufs=2, space="PSUM"))
            ps_ot = ctx.enter_context(tc.tile_pool(name="ps_ot", bufs=2, space="PSUM"))
            ps_st = ctx.enter_context(tc.tile_pool(name="ps_st", bufs=2, space="PSUM"))

            # ---- resident weights (bf16) -------------------------------
            def load_w(dram, label):
                tiles = []
                for kc in range(KC):
                    t = w_pool.tile([P, d_local], BF16, name=f"{label}{kc}")
                    nc.sync.dma_start(out=t, in_=dram[kc * P : (kc + 1) * P, :])
                    tiles.append(t)
                return tiles

            wq_sb = load_w(wqT, "wq")

            ones_t = ones_pool.tile([P, hd], BF16, name="ones_t")
            nc.vector.memset(ones_t, 1.0)
            warm_t = ones_pool.tile([hd, hd], BF16, name="warm_t")
            nc.vector.memset(warm_t, 1.0)

            warm = ps_a.tile([hd, hd], F32, name="warm", tag="psa")
            for _ in range(warmup):
                nc.tensor.matmul(warm, lhsT=warm_t, rhs=warm_t, start=True, stop=True)

            # ---- persistent attention operand tiles --------------------
            qt_sb = [
                [qt_pool.tile([P, cs], BF16, name=f"qt{mt}_{c}") for c in range(NQC)]
                for mt in range(DMT)
            ]
            kt_sb = [
                [kt_pool.tile([P, cs], BF16, name=f"kt{mt}_{c}") for c in range(NVC)]
                for mt in range(DMT)
            ]
            v_t = [
                v_pool.tile([P, n_heads, hd + 1], BF16, name=f"v{nt}")
                for nt in range(NKT)
            ]
            for nt in range(NKT):
                nc.gpsimd.memset(v_t[nt][:, :, hd : hd + 1], 1.0)

            # ---- projections -------------------------------------------
            def project_qk(x_dram, w_sb, out_tiles, label):
                for nch in range(NQC):
                    xch = []
                    for kc in range(KC):
                        t = xin_pool.tile([P, cs], BF16, name=f"x{label}{nch}_{kc}",
                                          tag="xin")
                        nc.sync.dma_start(
                            out=t,
                            in_=x_dram[kc * P : (kc + 1) * P,
                                       nch * cs : (nch + 1) * cs],
                        )
                        xch.append(t)
                    for mt in range(DMT):
                        ps = ps_a.tile([P, cs], F32, name=f"p{label}{nch}_{mt}",
                                       tag="psa")
                        for kc in range(KC):
                            nc.tensor.matmul(
                                ps,
                                lhsT=w_sb[kc][:, mt * P : (mt + 1) * P],
                                rhs=xch[kc],
                                start=(kc == 0),
                                stop=(kc == KC - 1),
                            )
                        nc.scalar.copy(out=out_tiles[mt][nch], in_=ps)

            project_qk(xqT, wq_sb, qt_sb, "q")
            wk_sb = load_w(wkT, "wk")
            project_qk(xkT, wk_sb, kt_sb, "k")
            wv_sb = load_w(wvT, "wv")

            for nch in range(NVC):
                xch = []
                for kc in range(KC):
                    t = xin_pool.tile([P, cs], BF16, name=f"xv{nch}_{kc}", tag="xin")
                    nc.sync.dma_start(
                        out=t,
                        in_=xvT[kc * P : (kc + 1) * P, nch * cs : (nch + 1) * cs],
                    )
                    xch.append(t)
                for sub in range(SUB):
                    ps = ps_a.tile([P, d_local], F32, name=f"pv{nch}_{sub}",
                                   tag="psa")
                    for kc in range(KC):
                        nc.tensor.matmul(
                            ps,
                            lhsT=xch[kc][:, sub * P : (sub + 1) * P],
                            rhs=wv_sb[kc],
                            start=(kc == 0),
                            stop=(kc == KC - 1),
                        )
                    nt = nch * SUB + sub
                    nc.scalar.copy(
                        out=v_t[nt][:, :, 0:hd],
                        in_=ps.rearrange("p (h d) -> p h d", h=n_heads),
                    )

            wo_sb = []
            for mt in range(DMT):
                t = w_pool.tile([P, odim], BF16, name=f"wo{mt}")
                nc.sync.dma_start(out=t, in_=woT[mt * P : (mt + 1) * P, :])
                wo_sb.append(t)

            # ---- attention ---------------------------------------------
            def emit_exp(ex, st, kt2):
                if kt2 % exp_dve_mod == exp_dve_mod - 1:
                    nc.vector.tensor_scalar(
                        out=ex.bitcast(I16),
                        in0=st,
                        scalar1=sch_a,
                        scalar2=SCH_C16,
                        op0=mybir.AluOpType.mult,
                        op1=mybir.AluOpType.add,
                    )
                else:
                    nc.scalar.activation(
                        out=ex,
                        in_=st,
                        func=mybir.ActivationFunctionType.Exp,
                        scale=scale,
                    )

            SKEW = 2

            def emit_outproj(qg_e, ot_e):
                qsl_e = slice(qg_e * cs, (qg_e + 1) * cs)
                for ob in range(OT):
                    po = ps_a.tile([P, cs], F32, name=f"po{qg_e}_{ob}", tag="psa")
                    for hpo in range(DMT):
                        nc.tensor.matmul(
                            po,
                            lhsT=wo_sb[hpo][:, ob * P : (ob + 1) * P],
                            rhs=ot_e[:, hpo, :],
                            start=(hpo == 0),
                            stop=(hpo == DMT - 1),
                        )
                    osb = out_pool.tile([P, cs], F32, name=f"osb{qg_e}_{ob}",
                                        tag="osb")
                    nc.vector.tensor_copy(out=osb, in_=po)
                    nc.sync.dma_start(
                        out=outT[ob * P : (ob + 1) * P, qsl_e], in_=osb
                    )

            prev_ot = None
            for qg in range(NQC):
                ot_t = ot_pool.tile([P, DMT, cs], BF16, name=f"ot{qg}", tag="ot")
                prev_norm = None

                def make_norm(ot_ps_n, heads_n, hp_n):
                    def norm():
                        for h in heads_n:
                            poff = (h % 2) * hd
                            raw = rcp_pool.tile([hd + 1, cs], BF16,
                                                name=f"raw{qg}_{h}", tag="raw")
                            nc.scalar.copy(out=raw, in_=ot_ps_n[h])
                            bcp = ps_a.tile([hd, cs], F32, name=f"bcp{qg}_{h}",
                                            tag="psa")
                            nc.tensor.matmul(
                                bcp,
                                lhsT=ones_t[hd : hd + 1, :],
                                rhs=raw[hd : hd + 1, :],
                                start=True,
                                stop=True,
                            )
                            bci = rcp_pool.tile([hd, cs], F32,
                                                name=f"bci{qg}_{h}", tag="bci")
                            nc.vector.reciprocal_approx_fast(out=bci, in_=bcp)
                            nc.vector.scalar_tensor_tensor(
                                out=ot_t[poff : poff + hd, hp_n, :],
                                in0=raw[0:hd, :],
                                scalar=1.0,
                                in1=bci,
                                op0=mybir.AluOpType.mult,
                                op1=mybir.AluOpType.mult,
                            )
                    return norm

                for hp in range(DMT):
                    heads = (2 * hp, 2 * hp + 1)
                    ot_ps = {
                        h: ps_ot.tile([hd + 1, cs], F32, name=f"otps{qg}_{h}",
                                      tag="psot")
                        for h in heads
                    }

                    def emit_pv(kt2_p, ex_p, heads_p=heads, ot_ps_p=ot_ps):
                        for jk in range(2):
                            kt = 2 * kt2_p + jk
                            for h in heads_p:
                                nc.tensor.matmul(
                                    ot_ps_p[h],
                                    lhsT=v_t[kt][:, h, :],
                                    rhs=ex_p[h][:, jk, :],
                                    start=(kt == 0),
                                    stop=(kt == NKT - 1),
                                )

                    pend = []
                    for kt2 in range(NKT // 2):
                        st = {
                            h: ps_st.tile([P, 2, cs], F32,
                                          name=f"st{qg}_{hp}_{kt2}_{h}", tag="st")
                            for h in heads
                        }
                        for jk in range(2):
                            kt = 2 * kt2 + jk
                            kch, sub = divmod(kt, SUB)
                            for h in heads:
                                poff = (h % 2) * hd
                                nc.tensor.matmul(
                                    st[h][:, jk, :],
                                    lhsT=kt_sb[hp][kch][
                                        poff : poff + hd,
                                        sub * P : (sub + 1) * P,
                                    ],
                                    rhs=qt_sb[hp][qg][poff : poff + hd, :],
                                    start=True,
                                    stop=True,
                                )
                        ex = {}
                        for h in heads:
                            ex[h] = ex_pool.tile([P, 2, cs], BF16,
                                                 name=f"ex{qg}_{kt2}_{h}", tag="ex")
                            emit_exp(ex[h], st[h], kt2)
                        if kt2 == 1 and prev_norm is not None:
                            prev_norm()
                        if kt2 == 3 and hp == 0 and prev_ot is not None:
                            emit_outproj(qg - 1, prev_ot)
                        pend.append((kt2, ex))
                        if len(pend) > SKEW:
                            emit_pv(*pend.pop(0))
                    for p in pend:
                        emit_pv(*p)
                    prev_norm = make_norm(ot_ps, heads, hp)
                prev_norm()
                prev_ot = ot_t
            emit_outproj(NQC - 1, prev_ot)
    nc.finalize()
    return nc


_NC_CACHE = {}


def _get_program(key, **kw):
    if key not in _NC_CACHE:
        _NC_CACHE[key] = build_core_program(**kw)
    return _NC_CACHE[key]


def _bf16(a):
    return np.ascontiguousarray(a.astype(NPBF16))


def kernel(xq, xk, xv, Wq, Wk, Wv, Wo, bo):
    m, nq, qkd = xq.shape
    nkv = xk.shape[1]
    inner = Wq.shape[0]
    odim = Wo.shape[0]
    assert (m, nq, qkd, nkv, inner, odim) == (4, 2048, 1024, 2048, 1024, 1024)
    n_cores = 8
    gheads = 2
    gslice = inner // gheads  # 512

    WqT = np.asarray(Wq, np.float32).T
    WkT = np.asarray(Wk, np.float32).T
    WvT = np.asarray(Wv, np.float32).T
    WoT = np.asarray(Wo, np.float32).T

    xq_b = [_bf16(np.asarray(xq[b], np.float32).T) for b in range(m)]
    xk_b = [_bf16(np.asarray(xk[b], np.float32).T) for b in range(m)]
    xv_b = [_bf16(np.asarray(xv[b], np.float32).T) for b in range(m)]

    in_maps = []
    for c in range(n_cores):
        b, g = divmod(c, gheads)
        sl = slice(g * gslice, (g + 1) * gslice)
        in_maps.append(
            {
                "xqT": xq_b[b],
                "xkT": xk_b[b],
                "xvT": xv_b[b],
                "wqT": _bf16(WqT[:, sl]),
                "wkT": _bf16(WkT[:, sl]),
                "wvT": _bf16(WvT[:, sl]),
                "woT": _bf16(WoT[sl, :]),
            }
        )

    nc = _get_program("full")
    res = run_bass_kernel_spmd(nc, in_maps, core_ids=list(range(n_cores)))
    global _LAST_RESULTS
    _LAST_RESULTS = res
    out = np.empty((m, nq, odim), np.float32)
    bo32 = np.asarray(bo, np.float32)[None, :]
    for b in range(m):
        acc = res.results[gheads * b]["outT"].copy()
        for g in range(1, gheads):
            acc += res.results[gheads * b + g]["outT"]
        out[b] = acc.T + bo32
    return out


# revision 5
# speedup vs baseline: 1.1011x; 1.0319x over previous
"""Multi-head attention Trainium2 Bass kernel, v2 (bf16 + 3-engine exp).

Structure follows the fp32r baseline (weights-stationary projections,
per-head S^T with K=64, ones-column PV for the softmax denominator,
row-parallel out-projection, host reduction over 2 head-groups) with:
  - all matmul operands bf16 (same 1 cycle/row PE rate as fp32r at
    N>=256, but half the LDWEIGHTS/DMA/SBUF traffic)
  - exp split across three engines: ACT (table exp, bf16 out) for 11/16
    of tiles, DVE and Pool computing a Schraudolph-style exp
    (int16 = s*183.87 + C, bitcast bf16) for the rest, so the scalar
    engine is never the attention-phase bottleneck
  - PSUM->SBUF projection-output quantize copies on the Pool engine
  - normalize via single-row reciprocal + PE broadcast of the
    reciprocal + one fused DVE multiply into bf16 ot tiles
  - short PE warmup (p-state ramp) overlapping the initial DMA
"""

import numpy as np
import ml_dtypes

import concourse.bass as bass
from concourse import bacc
import concourse.mybir as mybir
import concourse.tile as tile
from concourse.bass_utils import run_bass_kernel_spmd

F32 = mybir.dt.float32
F32R = mybir.dt.float32r
BF16 = mybir.dt.bfloat16
I16 = mybir.dt.int16
NPBF16 = ml_dtypes.bfloat16
P = 128

# exp split: kt2 % EXP_DVE_MOD == EXP_DVE_MOD-1 goes to DVE Schraudolph,
# the rest to ACT table exp (GPSIMD cannot read PSUM, so no Pool exp)
EXP_DVE_MOD = 4

# Schraudolph bf16: i16 = round(s_eff*(2^7/ln2) + C16); bitcast bf16
SCH_A16 = float(128.0 / np.log(2.0))
SCH_C16 = 16250.5  # tuned offline; adjust after HW comparison


def _install_drain_patch():
    import concourse.tile as _tile
    import concourse.mybir as _mybir
    from concourse.vector_clock import ScopedClock as _ScopedClock

    if getattr(_tile.TileContext, "_drain_split_patch", False):
        return

    def _patched_drain_and_barrier(self, tick_clock, wait_clock):
        drain_inst = self.nc.sync.drain()
        wait_clock.add_sem_waits(
            drain_inst.ins, _ScopedClock({None: tick_clock.global_clock})
        )
        si = drain_inst.ins.sync_info
        if si is not None and len(si.on_wait) > 1:
            waits = list(si.on_wait)
            drain_inst.ins.sync_info = _mybir.SyncInfo(
                on_wait=[waits[0]], on_update=list(si.on_update)
            )
            for w in waits[1:]:
                extra = self.nc.sync.drain()
                extra.ins.sync_info = _mybir.SyncInfo(on_wait=[w], on_update=[])
        self.nc.all_engine_barrier()
        assert self.sems is not None
        popped = self.nc._tile_sem_poison_stack.pop()
        assert popped is self._sem_poison
        self.nc.clear_and_free_semaphores(list(self.sems.allocated().values()))
        self.nc.all_engine_barrier()

    _tile.TileContext._drain_and_barrier = _patched_drain_and_barrier
    _tile.TileContext._drain_split_patch = True


_install_drain_patch()


def build_core_program(
    nq=2048,
    nkv=2048,
    ckdim=1024,
    n_heads=8,
    hd=64,
    odim=1024,
    cs=512,
    num_devices=8,
    warmup=130,
    exp_dve_mod=EXP_DVE_MOD,
):
    d_local = n_heads * hd  # 512
    scale = float(hd) ** -0.5
    sch_a = float(SCH_A16 * scale)
    KC = ckdim // P  # 8 contraction tiles for projections
    DMT = d_local // P  # 4 partition tiles (head pairs)
    NQC = nq // cs  # 4
    NVC = nkv // cs  # 4
    NKT = nkv // P  # 16 kv tiles
    OT = odim // P  # 8
    SUB = cs // P  # 4

    nc = bacc.Bacc(
        "TRN2", target_bir_lowering=False, debug=False, num_devices=num_devices
    )
    xqT = nc.declare_dram_parameter("xqT", [ckdim, nq], BF16, isOutput=False)
    xkT = nc.declare_dram_parameter("xkT", [ckdim, nkv], BF16, isOutput=False)
    xvT = nc.declare_dram_parameter("xvT", [ckdim, nkv], BF16, isOutput=False)
    wqT = nc.declare_dram_parameter("wqT", [ckdim, d_local], BF16, isOutput=False)
    wkT = nc.declare_dram_parameter("wkT", [ckdim, d_local], BF16, isOutput=False)
    wvT = nc.declare_dram_parameter("wvT", [ckdim, d_local], BF16, isOutput=False)
    woT = nc.declare_dram_parameter("woT", [d_local, odim], BF16, isOutput=False)
    outT = nc.declare_dram_parameter("outT", [odim, nq], F32, isOutput=True)

    with tile.TileContext(nc) as tc:
        import contextlib

        ctx = contextlib.ExitStack()
        with ctx:
            ctx.enter_context(
                nc.allow_low_precision(reason="bf16 pipeline, fp32 PSUM accum")
            )
            w_pool = ctx.enter_context(tc.tile_pool(name="w", bufs=1))
            xin_pool = ctx.enter_context(tc.tile_pool(name="xin", bufs=18))
            qt_pool = ctx.enter_context(tc.tile_pool(name="qt", bufs=1))
            kt_pool = ctx.enter_context(tc.tile_pool(name="kt", bufs=1))
            v_pool = ctx.enter_context(tc.tile_pool(name="v", bufs=1))
            ex_pool = ctx.enter_context(tc.tile_pool(name="expp", bufs=12))
            ot_pool = ctx.enter_context(tc.tile_pool(name="ot", bufs=2))
            rcp_pool = ctx.enter_context(tc.tile_pool(name="rcp", bufs=4))
            out_pool = ctx.enter_context(tc.tile_pool(name="outp", bufs=3))
            ones_pool = ctx.enter_context(tc.tile_pool(name="ones", bufs=1))
            ps_a = ctx.enter_context(tc.tile_pool(name="ps_a", bufs=2, space="PSUM"))
            ps_ot = ctx.enter_context(tc.tile_pool(name="ps_ot", bufs=2, space="PSUM"))
            ps_st = ctx.enter_context(tc.tile_pool(name="ps_st", bufs=2, space="PSUM"))

            # ---- resident weights (bf16) -------------------------------
            def load_w(dram, label):
                tiles = []
                for kc in range(KC):
                    t = w_pool.tile([P, d_local], BF16, name=f"{label}{kc}")
                    nc.sync.dma_start(out=t, in_=dram[kc * P : (kc + 1) * P, :])
                    tiles.append(t)
                return tiles

            wq_sb = load_w(wqT, "wq")

            ones_t = ones_pool.tile([P, hd], BF16, name="ones_t")
            nc.vector.memset(ones_t, 1.0)
            warm_t = ones_pool.tile([hd, hd], BF16, name="warm_t")
            nc.vector.memset(warm_t, 1.0)

            warm = ps_a.tile([hd, hd], F32, name="warm", tag="psa")
            for _ in range(warmup):
                nc.tensor.matmul(warm, lhsT=warm_t, rhs=warm_t, start=True, stop=True)

            # ---- persistent attention operand tiles --------------------
            qt_sb = [
                [qt_pool.tile([P, cs], BF16, name=f"qt{mt}_{c}") for c in range(NQC)]
                for mt in range(DMT)
            ]
            kt_sb = [
                [kt_pool.tile([P, cs], BF16, name=f"kt{mt}_{c}") for c in range(NVC)]
                for mt in range(DMT)
            ]
            v_t = [
                v_pool.tile([P, n_heads, hd + 1], BF16, name=f"v{nt}")
                for nt in range(NKT)
            ]
            for nt in range(NKT):
                nc.gpsimd.memset(v_t[nt][:, :, hd : hd + 1], 1.0)

            # ---- projections -------------------------------------------
            def project_qk(x_dram, w_sb, out_tiles, label):
                for nch in range(NQC):
                    xch = []
                    for kc in range(KC):
                        t = xin_pool.tile([P, cs], BF16, name=f"x{label}{nch}_{kc}",
                                          tag="xin")
                        nc.sync.dma_start(
                            out=t,
                            in_=x_dram[kc * P : (kc + 1) * P,
                                       nch * cs : (nch + 1) * cs],
                        )
                        xch.append(t)
                    for mt in range(DMT):
                        ps = ps_a.tile([P, cs], F32, name=f"p{label}{nch}_{mt}",
                                       tag="psa")
                        for kc in range(KC):
                            nc.tensor.matmul(
                                ps,
                                lhsT=w_sb[kc][:, mt * P : (mt + 1) * P],
                                rhs=xch[kc],
                                start=(kc == 0),
                                stop=(kc == KC - 1),
                            )
                        nc.scalar.copy(out=out_tiles[mt][nch], in_=ps)

            project_qk(xqT, wq_sb, qt_sb, "q")
            wk_sb = load_w(wkT, "wk")
            project_qk(xkT, wk_sb, kt_sb, "k")
            wv_sb = load_w(wvT, "wv")

            for nch in range(NVC):
                xch = []
                for kc in range(KC):
                    t = xin_pool.tile([P, cs], BF16, name=f"xv{nch}_{kc}", tag="xin")
                    nc.sync.dma_start(
                        out=t,
                        in_=xvT[kc * P : (kc + 1) * P, nch * cs : (nch + 1) * cs],
                    )
                    xch.append(t)
                for sub in range(SUB):
                    ps = ps_a.tile([P, d_local], F32, name=f"pv{nch}_{sub}",
                                   tag="psa")
                    for kc in range(KC):
                        nc.tensor.matmul(
                            ps,
                            lhsT=xch[kc][:, sub * P : (sub + 1) * P],
                            rhs=wv_sb[kc],
                            start=(kc == 0),
                            stop=(kc == KC - 1),
                        )
                    nt = nch * SUB + sub
                    nc.scalar.copy(
                        out=v_t[nt][:, :, 0:hd],
                        in_=ps.rearrange("p (h d) -> p h d", h=n_heads),
                    )

            wo_sb = []
            for mt in range(DMT):
                t = w_pool.tile([P, odim], BF16, name=f"wo{mt}")
                nc.sync.dma_start(out=t, in_=woT[mt * P : (mt + 1) * P, :])
                wo_sb.append(t)

            # ---- attention ---------------------------------------------
            def emit_exp(ex, st, kt2):
                if kt2 % exp_dve_mod == exp_dve_mod - 1:
                    nc.vector.tensor_scalar(
                        out=ex.bitcast(I16),
                        in0=st,
                        scalar1=sch_a,
                        scalar2=SCH_C16,
                        op0=mybir.AluOpType.mult,
                        op1=mybir.AluOpType.add,
                    )
                else:
                    nc.scalar.activation(
                        out=ex,
                        in_=st,
                        func=mybir.ActivationFunctionType.Exp,
                        scale=scale,
                    )

            SKEW = 2

            def emit_outproj(qg_e, ot_e, obs=None):
                qsl_e = slice(qg_e * cs, (qg_e + 1) * cs)
                for ob in (range(OT) if obs is None else obs):
                    po = ps_a.tile([P, cs], F32, name=f"po{qg_e}_{ob}", tag="psa")
                    for hpo in range(DMT):
                        nc.tensor.matmul(
                            po,
                            lhsT=wo_sb[hpo][:, ob * P : (ob + 1) * P],
                            rhs=ot_e[:, hpo, :],
                            start=(hpo == 0),
                            stop=(hpo == DMT - 1),
                        )
                    osb = out_pool.tile([P, cs], F32, name=f"osb{qg_e}_{ob}",
                                        tag="osb")
                    nc.vector.tensor_copy(out=osb, in_=po)
                    nc.sync.dma_start(
                        out=outT[ob * P : (ob + 1) * P, qsl_e], in_=osb
                    )

            prev_ot = None
            for qg in range(NQC):
                ot_t = ot_pool.tile([P, DMT, cs], BF16, name=f"ot{qg}", tag="ot")
                prev_norm = None

                def make_norm(ot_ps_n, heads_n, hp_n):
                    def norm():
                        for h in heads_n:
                            poff = (h % 2) * hd
                            raw = rcp_pool.tile([hd + 1, cs], BF16,
                                                name=f"raw{qg}_{h}", tag="raw")
                            nc.scalar.copy(out=raw, in_=ot_ps_n[h])
                            bcp = ps_a.tile([hd, cs], F32, name=f"bcp{qg}_{h}",
                                            tag="psa")
                            nc.tensor.matmul(
                                bcp,
                                lhsT=ones_t[hd : hd + 1, :],
                                rhs=raw[hd : hd + 1, :],
                                start=True,
                                stop=True,
                            )
                            bci = rcp_pool.tile([hd, cs], F32,
                                                name=f"bci{qg}_{h}", tag="bci")
                            nc.vector.reciprocal_approx_fast(out=bci, in_=bcp)
                            nc.vector.scalar_tensor_tensor(
                                out=ot_t[poff : poff + hd, hp_n, :],
                                in0=raw[0:hd, :],
                                scalar=1.0,
                                in1=bci,
                                op0=mybir.AluOpType.mult,
                                op1=mybir.AluOpType.mult,
                            )
                    return norm

                for hp in range(DMT):
                    heads = (2 * hp, 2 * hp + 1)
                    ot_ps = {
                        h: ps_ot.tile([hd + 1, cs], F32, name=f"otps{qg}_{h}",
                                      tag="psot")
                        for h in heads
                    }

                    def emit_pv(kt2_p, ex_p, heads_p=heads, ot_ps_p=ot_ps):
                        for jk in range(2):
                            kt = 2 * kt2_p + jk
                            for h in heads_p:
                                nc.tensor.matmul(
                                    ot_ps_p[h],
                                    lhsT=v_t[kt][:, h, :],
                                    rhs=ex_p[h][:, jk, :],
                                    start=(kt == 0),
                                    stop=(kt == NKT - 1),
                                )

                    pend = []
                    for kt2 in range(NKT // 2):
                        st = {
                            h: ps_st.tile([P, 2, cs], F32,
                                          name=f"st{qg}_{hp}_{kt2}_{h}", tag="st")
                            for h in heads
                        }
                        for jk in range(2):
                            kt = 2 * kt2 + jk
                            kch, sub = divmod(kt, SUB)
                            for h in heads:
                                poff = (h % 2) * hd
                                nc.tensor.matmul(
                                    st[h][:, jk, :],
                                    lhsT=kt_sb[hp][kch][
                                        poff : poff + hd,
                                        sub * P : (sub + 1) * P,
                                    ],
                                    rhs=qt_sb[hp][qg][poff : poff + hd, :],
                                    start=True,
                                    stop=True,
                                )
                        ex = {}
                        for h in heads:
                            ex[h] = ex_pool.tile([P, 2, cs], BF16,
                                                 name=f"ex{qg}_{kt2}_{h}", tag="ex")
                            emit_exp(ex[h], st[h], kt2)
                        if kt2 == 1 and prev_norm is not None:
                            prev_norm()
                        if kt2 == 3 and hp == 0 and prev_ot is not None:
                            emit_outproj(qg - 1, prev_ot, range(0, OT // 2))
                        if kt2 == 5 and hp == 0 and prev_ot is not None:
                            emit_outproj(qg - 1, prev_ot, range(OT // 2, OT))
                        pend.append((kt2, ex))
                        if len(pend) > SKEW:
                            emit_pv(*pend.pop(0))
                    for p in pend:
                        emit_pv(*p)
                    prev_norm = make_norm(ot_ps, heads, hp)
                prev_norm()
                prev_ot = ot_t
            emit_outproj(NQC - 1, prev_ot)
    nc.finalize()
    return nc


_NC_CACHE = {}


def _get_program(key, **kw):
    if key not in _NC_CACHE:
        _NC_CACHE[key] = build_core_program(**kw)
    return _NC_CACHE[key]


def _bf16(a):
    return np.ascontiguousarray(a.astype(NPBF16))


def kernel(xq, xk, xv, Wq, Wk, Wv, Wo, bo):
    m, nq, qkd = xq.shape
    nkv = xk.shape[1]
    inner = Wq.shape[0]
    odim = Wo.shape[0]
    assert (m, nq, qkd, nkv, inner, odim) == (4, 2048, 1024, 2048, 1024, 1024)
    n_cores = 8
    gheads = 2
    gslice = inner // gheads  # 512

    WqT = np.asarray(Wq, np.float32).T
    WkT = np.asarray(Wk, np.float32).T
    WvT = np.asarray(Wv, np.float32).T
    WoT = np.asarray(Wo, np.float32).T

    xq_b = [_bf16(np.asarray(xq[b], np.float32).T) for b in range(m)]
    xk_b = [_bf16(np.asarray(xk[b], np.float32).T) for b in range(m)]
    xv_b = [_bf16(np.asarray(xv[b], np.float32).T) for b in range(m)]

    in_maps = []
    for c in range(n_cores):
        b, g = divmod(c, gheads)
        sl = slice(g * gslice, (g + 1) * gslice)
        in_maps.append(
            {
                "xqT": xq_b[b],
                "xkT": xk_b[b],
                "xvT": xv_b[b],
                "wqT": _bf16(WqT[:, sl]),
                "wkT": _bf16(WkT[:, sl]),
                "wvT": _bf16(WvT[:, sl]),
                "woT": _bf16(WoT[sl, :]),
            }
        )

    nc = _get_program("full")
    res = run_bass_kernel_spmd(nc, in_maps, core_ids=list(range(n_cores)))
    global _LAST_RESULTS
    _LAST_RESULTS = res
    out = np.empty((m, nq, odim), np.float32)
    bo32 = np.asarray(bo, np.float32)[None, :]
    for b in range(m):
        acc = res.results[gheads * b]["outT"].copy()
        for g in range(1, gheads):
            acc += res.results[gheads * b + g]["outT"]
        out[b] = acc.T + bo32
    return out


# revision 6
# speedup vs baseline: 1.1066x; 1.0051x over previous
"""Multi-head attention Trainium2 Bass kernel, v2 (bf16 + 3-engine exp).

Structure follows the fp32r baseline (weights-stationary projections,
per-head S^T with K=64, ones-column PV for the softmax denominator,
row-parallel out-projection, host reduction over 2 head-groups) with:
  - all matmul operands bf16 (same 1 cycle/row PE rate as fp32r at
    N>=256, but half the LDWEIGHTS/DMA/SBUF traffic)
  - exp split across three engines: ACT (table exp, bf16 out) for 11/16
    of tiles, DVE and Pool computing a Schraudolph-style exp
    (int16 = s*183.87 + C, bitcast bf16) for the rest, so the scalar
    engine is never the attention-phase bottleneck
  - PSUM->SBUF projection-output quantize copies on the Pool engine
  - normalize via single-row reciprocal + PE broadcast of the
    reciprocal + one fused DVE multiply into bf16 ot tiles
  - short PE warmup (p-state ramp) overlapping the initial DMA
"""

import numpy as np
import ml_dtypes

import concourse.bass as bass
from concourse import bacc
import concourse.mybir as mybir
import concourse.tile as tile
from concourse.bass_utils import run_bass_kernel_spmd

F32 = mybir.dt.float32
F32R = mybir.dt.float32r
BF16 = mybir.dt.bfloat16
I16 = mybir.dt.int16
NPBF16 = ml_dtypes.bfloat16
P = 128

# exp split: kt2 % EXP_DVE_MOD == EXP_DVE_MOD-1 goes to DVE Schraudolph,
# the rest to ACT table exp (GPSIMD cannot read PSUM, so no Pool exp)
EXP_DVE_MOD = 4

# Schraudolph bf16: i16 = round(s_eff*(2^7/ln2) + C16); bitcast bf16
SCH_A16 = float(128.0 / np.log(2.0))
SCH_C16 = 16250.5  # tuned offline; adjust after HW comparison


def _install_drain_patch():
    import concourse.tile as _tile
    import concourse.mybir as _mybir
    from concourse.vector_clock import ScopedClock as _ScopedClock

    if getattr(_tile.TileContext, "_drain_split_patch", False):
        return

    def _patched_drain_and_barrier(self, tick_clock, wait_clock):
        drain_inst = self.nc.sync.drain()
        wait_clock.add_sem_waits(
            drain_inst.ins, _ScopedClock({None: tick_clock.global_clock})
        )
        si = drain_inst.ins.sync_info
        if si is not None and len(si.on_wait) > 1:
            waits = list(si.on_wait)
            drain_inst.ins.sync_info = _mybir.SyncInfo(
                on_wait=[waits[0]], on_update=list(si.on_update)
            )
            for w in waits[1:]:
                extra = self.nc.sync.drain()
                extra.ins.sync_info = _mybir.SyncInfo(on_wait=[w], on_update=[])
        self.nc.all_engine_barrier()
        assert self.sems is not None
        popped = self.nc._tile_sem_poison_stack.pop()
        assert popped is self._sem_poison
        self.nc.clear_and_free_semaphores(list(self.sems.allocated().values()))
        self.nc.all_engine_barrier()

    _tile.TileContext._drain_and_barrier = _patched_drain_and_barrier
    _tile.TileContext._drain_split_patch = True


_install_drain_patch()


def build_core_program(
    nq=2048,
    nkv=2048,
    ckdim=1024,
    n_heads=8,
    hd=64,
    odim=1024,
    cs=512,
    num_devices=8,
    warmup=130,
    exp_dve_mod=EXP_DVE_MOD,
):
    d_local = n_heads * hd  # 512
    scale = float(hd) ** -0.5
    sch_a = float(SCH_A16 * scale)
    KC = ckdim // P  # 8 contraction tiles for projections
    DMT = d_local // P  # 4 partition tiles (head pairs)
    NQC = nq // cs  # 4
    NVC = nkv // cs  # 4
    NKT = nkv // P  # 16 kv tiles
    OT = odim // P  # 8
    SUB = cs // P  # 4

    nc = bacc.Bacc(
        "TRN2", target_bir_lowering=False, debug=False, num_devices=num_devices
    )
    xqT = nc.declare_dram_parameter("xqT", [ckdim, nq], BF16, isOutput=False)
    xkT = nc.declare_dram_parameter("xkT", [ckdim, nkv], BF16, isOutput=False)
    xvT = nc.declare_dram_parameter("xvT", [ckdim, nkv], BF16, isOutput=False)
    wqT = nc.declare_dram_parameter("wqT", [ckdim, d_local], BF16, isOutput=False)
    wkT = nc.declare_dram_parameter("wkT", [ckdim, d_local], BF16, isOutput=False)
    wvT = nc.declare_dram_parameter("wvT", [ckdim, d_local], BF16, isOutput=False)
    woT = nc.declare_dram_parameter("woT", [d_local, odim], BF16, isOutput=False)
    outT = nc.declare_dram_parameter("outT", [odim, nq], F32, isOutput=True)

    with tile.TileContext(nc) as tc:
        import contextlib

        ctx = contextlib.ExitStack()
        with ctx:
            ctx.enter_context(
                nc.allow_low_precision(reason="bf16 pipeline, fp32 PSUM accum")
            )
            w_pool = ctx.enter_context(tc.tile_pool(name="w", bufs=1))
            xin_pool = ctx.enter_context(tc.tile_pool(name="xin", bufs=18))
            qt_pool = ctx.enter_context(tc.tile_pool(name="qt", bufs=1))
            kt_pool = ctx.enter_context(tc.tile_pool(name="kt", bufs=1))
            v_pool = ctx.enter_context(tc.tile_pool(name="v", bufs=1))
            ex_pool = ctx.enter_context(tc.tile_pool(name="expp", bufs=12))
            ot_pool = ctx.enter_context(tc.tile_pool(name="ot", bufs=2))
            rcp_pool = ctx.enter_context(tc.tile_pool(name="rcp", bufs=4))
            out_pool = ctx.enter_context(tc.tile_pool(name="outp", bufs=3))
            ones_pool = ctx.enter_context(tc.tile_pool(name="ones", bufs=1))
            ps_a = ctx.enter_context(tc.tile_pool(name="ps_a", bufs=2, space="PSUM"))
            ps_ot = ctx.enter_context(tc.tile_pool(name="ps_ot", bufs=2, space="PSUM"))
            ps_st = ctx.enter_context(tc.tile_pool(name="ps_st", bufs=2, space="PSUM"))

            # ---- resident weights (bf16) -------------------------------
            def load_w(dram, label):
                tiles = []
                for kc in range(KC):
                    t = w_pool.tile([P, d_local], BF16, name=f"{label}{kc}")
                    nc.sync.dma_start(out=t, in_=dram[kc * P : (kc + 1) * P, :])
                    tiles.append(t)
                return tiles

            wq_sb = load_w(wqT, "wq")

            ones_t = ones_pool.tile([P, hd], BF16, name="ones_t")
            nc.vector.memset(ones_t, 1.0)
            warm_t = ones_pool.tile([hd, hd], BF16, name="warm_t")
            nc.vector.memset(warm_t, 1.0)

            warm = ps_a.tile([hd, hd], F32, name="warm", tag="psa")
            for _ in range(warmup):
                nc.tensor.matmul(warm, lhsT=warm_t, rhs=warm_t, start=True, stop=True)

            # ---- persistent attention operand tiles --------------------
            qt_sb = [
                [qt_pool.tile([P, cs], BF16, name=f"qt{mt}_{c}") for c in range(NQC)]
                for mt in range(DMT)
            ]
            kt_sb = [
                [kt_pool.tile([P, cs], BF16, name=f"kt{mt}_{c}") for c in range(NVC)]
                for mt in range(DMT)
            ]
            v_t = [
                v_pool.tile([P, n_heads, hd + 1], BF16, name=f"v{nt}")
                for nt in range(NKT)
            ]
            for nt in range(NKT):
                nc.gpsimd.memset(v_t[nt][:, :, hd : hd + 1], 1.0)

            # ---- projections -------------------------------------------
            def project_qk(x_dram, w_sb, out_tiles, label):
                for nch in range(NQC):
                    xch = []
                    for kc in range(KC):
                        t = xin_pool.tile([P, cs], BF16, name=f"x{label}{nch}_{kc}",
                                          tag="xin")
                        nc.sync.dma_start(
                            out=t,
                            in_=x_dram[kc * P : (kc + 1) * P,
                                       nch * cs : (nch + 1) * cs],
                        )
                        xch.append(t)
                    for mt in range(DMT):
                        ps = ps_a.tile([P, cs], F32, name=f"p{label}{nch}_{mt}",
                                       tag="psa")
                        for kc in range(KC):
                            nc.tensor.matmul(
                                ps,
                                lhsT=w_sb[kc][:, mt * P : (mt + 1) * P],
                                rhs=xch[kc],
                                start=(kc == 0),
                                stop=(kc == KC - 1),
                            )
                        nc.scalar.copy(out=out_tiles[mt][nch], in_=ps)

            project_qk(xqT, wq_sb, qt_sb, "q")
            wk_sb = load_w(wkT, "wk")
            project_qk(xkT, wk_sb, kt_sb, "k")
            wv_sb = load_w(wvT, "wv")

            for nch in range(NVC):
                xch = []
                for kc in range(KC):
                    t = xin_pool.tile([P, cs], BF16, name=f"xv{nch}_{kc}", tag="xin")
                    nc.sync.dma_start(
                        out=t,
                        in_=xvT[kc * P : (kc + 1) * P, nch * cs : (nch + 1) * cs],
                    )
                    xch.append(t)
                for sub in range(SUB):
                    ps = ps_a.tile([P, d_local], F32, name=f"pv{nch}_{sub}",
                                   tag="psa")
                    for kc in range(KC):
                        nc.tensor.matmul(
                            ps,
                            lhsT=xch[kc][:, sub * P : (sub + 1) * P],
                            rhs=wv_sb[kc],
                            start=(kc == 0),
                            stop=(kc == KC - 1),
                        )
                    nt = nch * SUB + sub
                    nc.scalar.copy(
                        out=v_t[nt][:, :, 0:hd],
                        in_=ps.rearrange("p (h d) -> p h d", h=n_heads),
                    )

            wo_sb = []
            for mt in range(DMT):
                t = w_pool.tile([P, odim], BF16, name=f"wo{mt}")
                nc.sync.dma_start(out=t, in_=woT[mt * P : (mt + 1) * P, :])
                wo_sb.append(t)

            # ---- attention ---------------------------------------------
            def emit_exp(ex, st, kt2, h):
                if h % 2 == 1:
                    nc.vector.tensor_scalar(
                        out=ex.bitcast(I16),
                        in0=st,
                        scalar1=sch_a,
                        scalar2=SCH_C16,
                        op0=mybir.AluOpType.mult,
                        op1=mybir.AluOpType.add,
                    )
                else:
                    nc.scalar.activation(
                        out=ex,
                        in_=st,
                        func=mybir.ActivationFunctionType.Exp,
                        scale=scale,
                    )

            SKEW = 2

            def emit_outproj(qg_e, ot_e, obs=None):
                qsl_e = slice(qg_e * cs, (qg_e + 1) * cs)
                for ob in (range(OT) if obs is None else obs):
                    po = ps_a.tile([P, cs], F32, name=f"po{qg_e}_{ob}", tag="psa")
                    for hpo in range(DMT):
                        nc.tensor.matmul(
                            po,
                            lhsT=wo_sb[hpo][:, ob * P : (ob + 1) * P],
                            rhs=ot_e[:, hpo, :],
                            start=(hpo == 0),
                            stop=(hpo == DMT - 1),
                        )
                    osb = out_pool.tile([P, cs], F32, name=f"osb{qg_e}_{ob}",
                                        tag="osb")
                    nc.vector.tensor_copy(out=osb, in_=po)
                    nc.sync.dma_start(
                        out=outT[ob * P : (ob + 1) * P, qsl_e], in_=osb
                    )

            prev_ot = None
            for qg in range(NQC):
                ot_t = ot_pool.tile([P, DMT, cs], BF16, name=f"ot{qg}", tag="ot")
                prev_norm = None

                def make_norm(ot_ps_n, heads_n, hp_n):
                    def norm():
                        for h in heads_n:
                            poff = (h % 2) * hd
                            raw = rcp_pool.tile([hd + 1, cs], BF16,
                                                name=f"raw{qg}_{h}", tag="raw")
                            nc.scalar.copy(out=raw, in_=ot_ps_n[h])
                            bcp = ps_a.tile([hd, cs], F32, name=f"bcp{qg}_{h}",
                                            tag="psa")
                            nc.tensor.matmul(
                                bcp,
                                lhsT=ones_t[hd : hd + 1, :],
                                rhs=raw[hd : hd + 1, :],
                                start=True,
                                stop=True,
                            )
                            bci = rcp_pool.tile([hd, cs], F32,
                                                name=f"bci{qg}_{h}", tag="bci")
                            nc.vector.reciprocal_approx_fast(out=bci, in_=bcp)
                            nc.vector.scalar_tensor_tensor(
                                out=ot_t[poff : poff + hd, hp_n, :],
                                in0=raw[0:hd, :],
                                scalar=1.0,
                                in1=bci,
                                op0=mybir.AluOpType.mult,
                                op1=mybir.AluOpType.mult,
                            )
                    return norm

                for hp in range(DMT):
                    heads = (2 * hp, 2 * hp + 1)
                    ot_ps = {
                        h: ps_ot.tile([hd + 1, cs], F32, name=f"otps{qg}_{h}",
                                      tag="psot")
                        for h in heads
                    }

                    def emit_pv(kt2_p, ex_p, heads_p=heads, ot_ps_p=ot_ps):
                        for jk in range(2):
                            kt = 2 * kt2_p + jk
                            for h in heads_p:
                                nc.tensor.matmul(
                                    ot_ps_p[h],
                                    lhsT=v_t[kt][:, h, :],
                                    rhs=ex_p[h][:, jk, :],
                                    start=(kt == 0),
                                    stop=(kt == NKT - 1),
                                )

                    pend = []
                    for kt2 in range(NKT // 2):
                        st = {
                            h: ps_st.tile([P, 2, cs], F32,
                                          name=f"st{qg}_{hp}_{kt2}_{h}", tag="st")
                            for h in heads
                        }
                        for jk in range(2):
                            kt = 2 * kt2 + jk
                            kch, sub = divmod(kt, SUB)
                            for h in heads:
                                poff = (h % 2) * hd
                                nc.tensor.matmul(
                                    st[h][:, jk, :],
                                    lhsT=kt_sb[hp][kch][
                                        poff : poff + hd,
                                        sub * P : (sub + 1) * P,
                                    ],
                                    rhs=qt_sb[hp][qg][poff : poff + hd, :],
                                    start=True,
                                    stop=True,
                                )
                        ex = {}
                        for h in heads:
                            ex[h] = ex_pool.tile([P, 2, cs], BF16,
                                                 name=f"ex{qg}_{kt2}_{h}", tag="ex")
                            emit_exp(ex[h], st[h], kt2, h)
                        if kt2 == 1 and prev_norm is not None:
                            prev_norm()
                        if kt2 == 3 and hp == 0 and prev_ot is not None:
                            emit_outproj(qg - 1, prev_ot, range(0, OT // 2))
                        if kt2 == 5 and hp == 0 and prev_ot is not None:
                            emit_outproj(qg - 1, prev_ot, range(OT // 2, OT))
                        pend.append((kt2, ex))
                        if len(pend) > SKEW:
                            emit_pv(*pend.pop(0))
                    for p in pend:
                        emit_pv(*p)
                    prev_norm = make_norm(ot_ps, heads, hp)
                prev_norm()
                prev_ot = ot_t
            emit_outproj(NQC - 1, prev_ot)
    nc.finalize()
    return nc


_NC_CACHE = {}


def _get_program(key, **kw):
    if key not in _NC_CACHE:
        _NC_CACHE[key] = build_core_program(**kw)
    return _NC_CACHE[key]


def _bf16(a):
    return np.ascontiguousarray(a.astype(NPBF16))


def kernel(xq, xk, xv, Wq, Wk, Wv, Wo, bo):
    m, nq, qkd = xq.shape
    nkv = xk.shape[1]
    inner = Wq.shape[0]
    odim = Wo.shape[0]
    assert (m, nq, qkd, nkv, inner, odim) == (4, 2048, 1024, 2048, 1024, 1024)
    n_cores = 8
    gheads = 2
    gslice = inner // gheads  # 512

    WqT = np.asarray(Wq, np.float32).T
    WkT = np.asarray(Wk, np.float32).T
    WvT = np.asarray(Wv, np.float32).T
    WoT = np.asarray(Wo, np.float32).T

    xq_b = [_bf16(np.asarray(xq[b], np.float32).T) for b in range(m)]
    xk_b = [_bf16(np.asarray(xk[b], np.float32).T) for b in range(m)]
    xv_b = [_bf16(np.asarray(xv[b], np.float32).T) for b in range(m)]

    in_maps = []
    for c in range(n_cores):
        b, g = divmod(c, gheads)
        sl = slice(g * gslice, (g + 1) * gslice)
        in_maps.append(
            {
                "xqT": xq_b[b],
                "xkT": xk_b[b],
                "xvT": xv_b[b],
                "wqT": _bf16(WqT[:, sl]),
                "wkT": _bf16(WkT[:, sl]),
                "wvT": _bf16(WvT[:, sl]),
                "woT": _bf16(WoT[sl, :]),
            }
        )

    nc = _get_program("full")
    res = run_bass_kernel_spmd(nc, in_maps, core_ids=list(range(n_cores)))
    global _LAST_RESULTS
    _LAST_RESULTS = res
    out = np.empty((m, nq, odim), np.float32)
    bo32 = np.asarray(bo, np.float32)[None, :]
    for b in range(m):
        acc = res.results[gheads * b]["outT"].copy()
        for g in range(1, gheads):
            acc += res.results[gheads * b + g]["outT"]
        out[b] = acc.T + bo32
    return out
